# revision 8
# baseline (speedup 1.0000x reference)
"""Causal self-attention (B=2,T=2048,C=1024,H=16) on 8 trn2 NeuronCores.

Sharding: tensor-parallel over heads for QKV+attention (2 heads/core),
AllToAll to switch to row-sharding, then row-parallel output projection.
Each core returns its 512-row block of the output; host concatenates.

Layouts (per core c):
  xT      [1024, 4096]   xT[ch, b*2048+t] = x[b,t,ch]      (same on all cores)
  wqT/wkT/wvT [1024,128] W.T columns for heads 2c,2c+1
  wpT     [1024, 1024]   Wp.T                               (same on all cores)
  q/k computed transposed qT/kT [128feat, 2048] per batch; v natural [T, 64]
  scores computed transposed ST [s, q]; softmax denominator via a ones-column
  appended to the AV weights (lhsT = [v | 1], M=65); exp folds the 1/sqrt(HD)
  scale; causal mask via gpsimd affine_select on diagonal chunks.
"""

from contextlib import ExitStack

import numpy as np

import concourse.bass as bass
import concourse.mybir as mybir
import concourse.tile as tile
from concourse import bacc
from concourse.bass_utils import run_bass_kernel_spmd
from concourse.masks import make_identity

B, T, C, H, HD = 2, 2048, 1024, 16, 64
NCORES = 8
HPC = H // NCORES          # heads per core = 2
F = HPC * HD               # per-core qkv feature count = 128
RPC = B * T // NCORES      # output rows per core = 512
NKT = C // 128             # contraction tiles = 8
NTB = T // 512             # T blocks per batch = 4
NST = T // 128             # s-tiles per batch = 16

FP32 = mybir.dt.float32
FP32R = mybir.dt.float32r
SCALE = 1.0 / float(np.sqrt(HD))


def _r(ap):
    """View an AP as float32r for full-rate PE matmuls."""
    return ap.bitcast(FP32R)


def build_nc():
    nc = bacc.Bacc("TRN2", target_bir_lowering=False, debug=False,
                   num_devices=NCORES)

    xT = nc.dram_tensor("xT", [C, B * T], FP32R, kind="ExternalInput").ap()
    wqT = nc.dram_tensor("wqT", [C, F], FP32R, kind="ExternalInput").ap()
    wkT = nc.dram_tensor("wkT", [C, F], FP32R, kind="ExternalInput").ap()
    wvT = nc.dram_tensor("wvT", [C, F], FP32R, kind="ExternalInput").ap()
    wpT = nc.dram_tensor("wpT", [C, C], FP32R, kind="ExternalInput").ap()
    bq = nc.dram_tensor("bq", [F, 1], FP32, kind="ExternalInput").ap()
    bk = nc.dram_tensor("bk", [F, 1], FP32, kind="ExternalInput").ap()
    bv = nc.dram_tensor("bv", [F, 1], FP32, kind="ExternalInput").ap()
    bp_rep = nc.dram_tensor("bp_rep", [128, C], FP32, kind="ExternalInput").ap()
    mask = nc.dram_tensor("mask", [128, 2048], FP32, kind="ExternalInput").ap()
    out = nc.dram_tensor("out", [RPC, C], FP32, kind="ExternalOutput").ap()

    with tile.TileContext(nc) as tc, ExitStack() as ctx:
        # ---- persistent SBUF ----
        persist = ctx.enter_context(tc.tile_pool(name="persist", bufs=1))
        wq_sb = persist.tile([128, C], FP32R, tag="wq")       # [cpart, ktile*F]
        wk_sb = persist.tile([128, C], FP32R, tag="wk")
        wv_sb = persist.tile([128, C], FP32R, tag="wv")
        wp_sb = persist.tile([128, NKT * C], FP32R, tag="wp")  # [cpart, ktile*C]
        bq_sb = persist.tile([F, 1], FP32, tag="bqt")
        bk_sb = persist.tile([F, 1], FP32, tag="bkt")
        bv_sb = persist.tile([F, 1], FP32, tag="bvt")
        bp_sb = persist.tile([128, C], FP32, tag="bpt")
        mask_sb = persist.tile([128, 2048], FP32, tag="mask")
        id_sb = persist.tile([128, 128], FP32, tag="idt")
        yf_sb = persist.tile([128, NCORES * RPC], FP32R, tag="yf")

        for k in range(NKT):
            nc.sync.dma_start(wq_sb[:, k * F:(k + 1) * F], wqT[k * 128:(k + 1) * 128, :])
            nc.sync.dma_start(wk_sb[:, k * F:(k + 1) * F], wkT[k * 128:(k + 1) * 128, :])
            nc.sync.dma_start(wv_sb[:, k * F:(k + 1) * F], wvT[k * 128:(k + 1) * 128, :])
            nc.sync.dma_start(wp_sb[:, k * C:(k + 1) * C], wpT[k * 128:(k + 1) * 128, :])
        nc.sync.dma_start(bq_sb[:], bq[:])
        nc.sync.dma_start(bk_sb[:], bk[:])
        nc.sync.dma_start(bv_sb[:], bv[:])
        nc.sync.dma_start(bp_sb[:], bp_rep[:])
        nc.sync.dma_start(mask_sb[:], mask[:])
        make_identity(nc, id_sb[:])

        # ---- DRAM staging for the AllToAll ----
        dram = ctx.enter_context(tc.tile_pool(name="dram", bufs=1, space="DRAM"))
        a2a_in = dram.tile([NCORES, F, RPC], FP32R)
        a2a_out = dram.tile([NCORES, F, RPC], FP32R)

        # ---- pools ----
        qkT_pool = ctx.enter_context(tc.tile_pool(name="qkT", bufs=2))
        v_pool = ctx.enter_context(tc.tile_pool(name="vsb", bufs=2))
        xt_pool = ctx.enter_context(tc.tile_pool(name="xt", bufs=12))
        vtmp_pool = ctx.enter_context(tc.tile_pool(name="vtmp", bufs=2))
        ex_pool = ctx.enter_context(tc.tile_pool(name="ex", bufs=4))
        sm_pool = ctx.enter_context(tc.tile_pool(name="sm", bufs=2))
        yout_pool = ctx.enter_context(tc.tile_pool(name="yout", bufs=2))

        ps_qkv = ctx.enter_context(tc.tile_pool(name="ps_qkv", bufs=2, space="PSUM"))
        ps_st = ctx.enter_context(tc.tile_pool(name="ps_st", bufs=2, space="PSUM"))
        ps_av = ctx.enter_context(tc.tile_pool(name="ps_av", bufs=2, space="PSUM"))

        VSTRIDE = HD + 1  # 65: v tile plus ones column

        for b in range(B):
            # ================= QKV projections for batch b =================
            qT_sb = qkT_pool.tile([128, T], FP32R, tag="qT")
            kT_sb = qkT_pool.tile([128, T], FP32R, tag="kT")
            v_sb = v_pool.tile([128, HPC * NST * VSTRIDE], FP32R, tag="v")
            nc.gpsimd.memset(v_sb[:].bitcast(FP32), 1.0)

            for tb in range(NTB):
                xts = []
                for k in range(NKT):
                    xt = xt_pool.tile([128, 512], FP32R, tag="xt")
                    nc.sync.dma_start(
                        xt[:], xT[k * 128:(k + 1) * 128,
                                  b * T + tb * 512: b * T + (tb + 1) * 512])
                    xts.append(xt)

                for w_sb, bias_sb, dest in ((wq_sb, bq_sb, "q"), (wk_sb, bk_sb, "k"),
                                            (wv_sb, bv_sb, "v")):
                    ps = ps_qkv.tile([128, 512], FP32, tag="ps")
                    for k in range(NKT):
                        nc.tensor.matmul(
                            ps[:], w_sb[:, k * F:(k + 1) * F], xts[k][:],
                            start=(k == 0), stop=(k == NKT - 1))
                    if dest == "q":
                        nc.vector.tensor_scalar_add(
                            qT_sb[:, tb * 512:(tb + 1) * 512], ps[:], bias_sb[:, 0:1])
                    elif dest == "k":
                        nc.vector.tensor_scalar_add(
                            kT_sb[:, tb * 512:(tb + 1) * 512], ps[:], bias_sb[:, 0:1])
                    else:
                        vtmp = vtmp_pool.tile([128, 512], FP32, tag="vtmp")
                        nc.vector.tensor_scalar_add(vtmp[:], ps[:], bias_sb[:, 0:1])
                        # transpose [feat, t] -> [t, feat]; split heads into v_sb
                        for u in range(4):
                            si = tb * 4 + u
                            pt = ps_qkv.tile([128, 128], FP32, tag="ps")
                            nc.tensor.transpose(pt[:], vtmp[:, u * 128:(u + 1) * 128],
                                                id_sb[:])
                            for hh in range(HPC):
                                base = (hh * NST + si) * VSTRIDE
                                nc.vector.tensor_copy(
                                    v_sb[:, base:base + HD],
                                    pt[:, hh * HD:(hh + 1) * HD])

            # ================= attention for batch b =================
            for j in range(NTB):  # q blocks of 512
                q0 = j * 512
                nch = 2 * (j + 1)  # chunks of 256 s values (2 s-tiles)
                avs = [ps_av.tile([HD + 1, 512], FP32, tag="av", name=f"av{b}_{j}_{_hh}")
                       for _hh in range(HPC)]
                for ch in range(nch):
                    s0 = ch * 256
                    for hh in range(HPC):
                        st = ps_st.tile([128, 1024], FP32, tag="st")
                        for k in range(2):
                            nc.tensor.matmul(
                                st[:, k * 512:(k + 1) * 512],
                                kT_sb[hh * HD:(hh + 1) * HD,
                                      s0 + k * 128: s0 + (k + 1) * 128],
                                qT_sb[hh * HD:(hh + 1) * HD, q0:q0 + 512],
                                start=True, stop=True)
                        ex = ex_pool.tile([128, 1024], FP32R, tag="ex")
                        if s0 >= q0:  # diagonal chunk: mask after exp
                            h = (s0 - q0) // 256
                            ex32 = ex_pool.tile([128, 1024], FP32, tag="ex32")
                            nc.scalar.activation(ex32[:], st[:],
                                                 mybir.ActivationFunctionType.Exp,
                                                 scale=SCALE)
                            nc.vector.tensor_mul(
                                ex[:], ex32[:], mask_sb[:, h * 1024:(h + 1) * 1024])
                        else:
                            nc.scalar.activation(ex[:], st[:],
                                                 mybir.ActivationFunctionType.Exp,
                                                 scale=SCALE)
                        for k in range(2):
                            si = (s0 + k * 128) // 128
                            vbase = (hh * NST + si) * VSTRIDE
                            nc.tensor.matmul(
                                avs[hh][:],
                                v_sb[:, vbase:vbase + VSTRIDE],
                                ex[:, k * 512:(k + 1) * 512],
                                start=(ch == 0 and k == 0),
                                stop=(ch == nch - 1 and k == 1))
                # normalize and stage for AllToAll
                for hh in range(HPC):
                    rec = sm_pool.tile([1, 512], FP32, tag="rec")
                    nc.vector.reciprocal(rec[:], avs[hh][HD:HD + 1, :])
                    rbc = sm_pool.tile([HD, 512], FP32, tag="rbc")
                    nc.gpsimd.partition_broadcast(rbc[:], rec[:], channels=HD)
                    yn = sm_pool.tile([HD, 512], FP32R, tag="yn")
                    nc.vector.tensor_mul(yn[:], avs[hh][0:HD, :], rbc[:])
                    nc.sync.dma_start(
                        a2a_in[b * NTB + j, hh * HD:(hh + 1) * HD, :], yn[:])

        # ================= AllToAll: head-shard -> row-shard =================
        nc.gpsimd.collective_compute(
            "AllToAll", mybir.AluOpType.bypass,
            replica_groups=[list(range(NCORES))],
            ins=[a2a_in.opt()], outs=[a2a_out.opt()])

        # ================= output projection (rows 512c .. 512c+512) ========
        for i in range(NCORES):
            nc.sync.dma_start(yf_sb[:, i * RPC:(i + 1) * RPC], a2a_out[i, :, :])
        for r in range(RPC // 128):
            for fb in range(C // 512):
                ps = ps_qkv.tile([128, 512], FP32, tag="ps")
                for i in range(NCORES):
                    nc.tensor.matmul(
                        ps[:],
                        yf_sb[:, i * RPC + r * 128: i * RPC + (r + 1) * 128],
                        wp_sb[:, i * C + fb * 512: i * C + (fb + 1) * 512],
                        start=(i == 0), stop=(i == NCORES - 1))
                ob = yout_pool.tile([128, 512], FP32, tag="ob")
                nc.vector.tensor_add(ob[:], ps[:], bp_sb[:, fb * 512:(fb + 1) * 512])
                nc.sync.dma_start(
                    out[r * 128:(r + 1) * 128, fb * 512:(fb + 1) * 512], ob[:])

    nc.compile()
    return nc


def make_in_maps(x, Wq, bq, Wk, bk, Wv, bv, Wp, bp):
    x = np.asarray(x, dtype=np.float32)
    Wq, Wk, Wv, Wp = (np.asarray(w, dtype=np.float32) for w in (Wq, Wk, Wv, Wp))
    bq, bk, bv, bp = (np.asarray(v, dtype=np.float32) for v in (bq, bk, bv, bp))

    xT = np.ascontiguousarray(x.transpose(2, 0, 1).reshape(C, B * T))
    wqT_full = np.ascontiguousarray(Wq.T)
    wkT_full = np.ascontiguousarray(Wk.T)
    wvT_full = np.ascontiguousarray(Wv.T)
    wpT = np.ascontiguousarray(Wp.T)
    bp_rep = np.ascontiguousarray(np.broadcast_to(bp, (128, C)))
    ii = np.arange(128)[:, None]
    jj = np.arange(512)[None, :]
    mask = np.zeros((128, 2048), np.float32)
    for h in range(2):
        for t in range(2):
            blk = (jj >= 256 * h + 128 * t + ii).astype(np.float32)
            mask[:, 1024 * h + 512 * t: 1024 * h + 512 * (t + 1)] = blk

    in_maps = []
    for c in range(NCORES):
        lo, hi = c * F, (c + 1) * F
        in_maps.append({
            "xT": xT,
            "wqT": np.ascontiguousarray(wqT_full[:, lo:hi]),
            "wkT": np.ascontiguousarray(wkT_full[:, lo:hi]),
            "wvT": np.ascontiguousarray(wvT_full[:, lo:hi]),
            "wpT": wpT,
            "bq": np.ascontiguousarray(bq[lo:hi, None]),
            "bk": np.ascontiguousarray(bk[lo:hi, None]),
            "bv": np.ascontiguousarray(bv[lo:hi, None]),
            "bp_rep": bp_rep,
            "mask": mask,
        })
    return in_maps


_NC_CACHE = {}


def kernel(x, Wq, bq, Wk, bk, Wv, bv, Wp, bp):
    if "nc" not in _NC_CACHE:
        _NC_CACHE["nc"] = build_nc()
    nc = _NC_CACHE["nc"]
    in_maps = make_in_maps(x, Wq, bq, Wk, bk, Wv, bv, Wp, bp)
    res = run_bass_kernel_spmd(nc, in_maps, list(range(NCORES)))
    blocks = [res.results[j]["out"] for j in range(NCORES)]
    return np.concatenate(blocks, axis=0).reshape(B, T, C)
